# revision 35
# baseline (speedup 1.0000x reference)
import math
import sys

sys.path.insert(0, "/opt/trn_rl_repo")

import numpy as np

# ---- model constants (from the reference nn.Module) ----
ROPE_PERIOD = 19.0
OMEGA = 2.0 * math.pi / ROPE_PERIOD
PEAK_EPS = 0.3
TARGET_LOGIT_GAP = math.log(10.0)
ATTN_AMPLITUDE = TARGET_LOGIT_GAP / (
    math.cos(OMEGA * PEAK_EPS) - math.cos(OMEGA * (1.0 - PEAK_EPS))
)
QK_NORM_SCALE = math.sqrt(ATTN_AMPLITUDE / math.sqrt(2.0))
SCALE = 2.0 ** (-0.5) * QK_NORM_SCALE**2
EMBED_CONST = 1000.0
EPS = 1e-6

B, L = 4, 4096
N_CORES = 8
SQRT2 = math.sqrt(2.0)

# Query-chunk split across the core pair of each batch: both sets cost 36
# causal tile-pairs, so no cross-core N/D reduction is needed. The SPMD
# instruction stream pads each slot to P pairs; surplus tiles are zeroed
# by the host-staged mask blob.
CHUNKS = [[0, 3, 4, 7], [1, 2, 5, 6]]
PPAIRS = [4, 8, 12, 16]        # static pairs per slot (max over parities)
NMASK = 4                      # masked pairs at the tail of each slot

# product-block pattern for the triple-bf16-split logits matmul:
# logit = sum over (a,b) in {(0,0),(1,0),(0,1),(2,0),(1,1),(0,2)} of q_a.k_b
QLVL = [0, 1, 0, 2, 1, 0]
KLVL = [0, 0, 1, 0, 1, 2]

_compiled = None


def _build():
    import concourse.bass as bass
    import concourse.tile as tile
    from concourse import bacc, mybir

    f32 = mybir.dt.float32
    f32r = mybir.dt.float32r
    bf16 = mybir.dt.bfloat16
    AF = mybir.ActivationFunctionType
    OP = mybir.AluOpType

    nc = bacc.Bacc("TRN2", target_bir_lowering=False, debug=False,
                   num_devices=N_CORES)

    # per-core inputs (host pre-arranged layouts; pure indexing, no math)
    xq_d = nc.dram_tensor("xq", [16, 256], f32, kind="ExternalInput").ap()
    tq_d = nc.dram_tensor("tq", [16, 256], f32, kind="ExternalInput").ap()
    xq2_d = nc.dram_tensor("xq2", [128, 32], f32, kind="ExternalInput").ap()
    xk_d = nc.dram_tensor("xk", [32, 256], f32, kind="ExternalInput").ap()
    tk_d = nc.dram_tensor("tk", [32, 256], f32, kind="ExternalInput").ap()
    xkb_d = nc.dram_tensor("xkb", [128, 64], f32, kind="ExternalInput").ap()
    mask_d = nc.dram_tensor("masks", [128, 16 * 1024], mybir.dt.bfloat16,
                            kind="ExternalInput").ap()
    wq_d = nc.dram_tensor("wq", [1], f32, kind="ExternalInput").ap()
    wv_d = nc.dram_tensor("wv", [1], f32, kind="ExternalInput").ap()
    wg_d = nc.dram_tensor("wg", [2], f32, kind="ExternalInput").ap()
    wc_d = nc.dram_tensor("wc", [1], f32, kind="ExternalInput").ap()
    out_d = nc.dram_tensor("out", [128, 32], f32, kind="ExternalOutput").ap()

    with tile.TileContext(nc) as tc:
        with (
            tc.tile_pool(name="const", bufs=1) as cp,
            tc.tile_pool(name="work", bufs=2) as wp,
            tc.tile_pool(name="ep", bufs=6) as ep,
            tc.tile_pool(name="pslog", bufs=3, space="PSUM") as pslog,
            tc.tile_pool(name="psnd", bufs=1, space="PSUM") as psnd,
            tc.tile_pool(name="psw", bufs=1, space="PSUM") as psw,
        ):
            # ---------- load inputs ----------
            xq = cp.tile([16, 256], f32, tag="xq")
            nc.sync.dma_start(xq[:], xq_d[:])
            tq = cp.tile([16, 256], f32, tag="tq")
            nc.sync.dma_start(tq[:], tq_d[:])
            xq2 = cp.tile([128, 32], f32, tag="xq2")
            nc.sync.dma_start(xq2[:], xq2_d[:])
            xk = cp.tile([32, 256], f32, tag="xk")
            nc.sync.dma_start(xk[:], xk_d[:])
            tk = cp.tile([32, 256], f32, tag="tk")
            nc.sync.dma_start(tk[:], tk_d[:])
            xkb = cp.tile([128, 64], f32, tag="xkb")
            nc.sync.dma_start(xkb[:], xkb_d[:])
            masks = cp.tile([128, 16 * 1024], bf16, tag="masks")
            nc.sync.dma_start(masks[:], mask_d[:])

            # PE warm-up: keep the tensor engine busy during the prologue so
            # HAM/p-state are ramped by the time the first logits matmul
            # issues. Dummy bf16 matmuls on a zeroed tile.
            dummy = cp.tile([128, 512], bf16, tag="dummy")
            nc.gpsimd.memset(dummy[:], 0.0)
            for _ in range(16):
                wps = psw.tile([128, 512], f32, tag="wps", name="wps")
                nc.tensor.matmul(wps[:], dummy[:, 0:128], dummy[:],
                                 start=True, stop=True)

            sw = cp.tile([1, 8], f32, tag="sw")  # scalar workspace row
            nc.sync.dma_start(sw[0:1, 0:1], wq_d[0:1])
            nc.sync.dma_start(sw[0:1, 1:2], wv_d[0:1])
            nc.sync.dma_start(sw[0:1, 2:4], wg_d[0:2])
            nc.sync.dma_start(sw[0:1, 4:5], wc_d[0:1])

            # ---------- scalar prep: cos/sin(phi) by Taylor, gate consts ----
            # layout of wvec [1, 8]: 0=cS 1=sS 2=wv 3=ga 4=gc 5=ga2 6=wc
            wvec = cp.tile([1, 8], f32, tag="wvec")
            t2 = cp.tile([1, 1], f32, tag="t2")
            nc.vector.tensor_tensor(t2[:], sw[0:1, 0:1], sw[0:1, 0:1], OP.mult)
            u = cp.tile([1, 1], f32, tag="u")
            nc.vector.tensor_scalar(u[:], t2[:], -1.0 / 720.0, 1.0 / 24.0,
                                    OP.mult, OP.add)
            nc.vector.tensor_scalar(u[:], u[:], t2[0:1, 0:1], -0.5,
                                    OP.mult, OP.add)
            nc.vector.tensor_scalar(u[:], u[:], t2[0:1, 0:1], 1.0,
                                    OP.mult, OP.add)
            nc.vector.tensor_scalar(wvec[0:1, 0:1], u[:], SCALE, None, OP.mult)
            v = cp.tile([1, 1], f32, tag="v")
            nc.vector.tensor_scalar(v[:], t2[:], -1.0 / 5040.0, 1.0 / 120.0,
                                    OP.mult, OP.add)
            nc.vector.tensor_scalar(v[:], v[:], t2[0:1, 0:1], -1.0 / 6.0,
                                    OP.mult, OP.add)
            nc.vector.tensor_scalar(v[:], v[:], t2[0:1, 0:1], 1.0,
                                    OP.mult, OP.add)
            nc.vector.tensor_scalar(v[:], v[:], sw[0:1, 0:1], None, OP.mult)
            nc.vector.tensor_scalar(wvec[0:1, 1:2], v[:], SCALE, None, OP.mult)
            nc.vector.tensor_copy(wvec[0:1, 2:3], sw[0:1, 1:2])
            nc.vector.tensor_copy(wvec[0:1, 3:4], sw[0:1, 2:3])
            nc.vector.tensor_copy(wvec[0:1, 4:5], sw[0:1, 3:4])
            nc.vector.tensor_scalar(wvec[0:1, 5:6], sw[0:1, 3:4],
                                    -1.0 / EMBED_CONST, sw[0:1, 2:3],
                                    OP.mult, OP.add)
            nc.vector.tensor_copy(wvec[0:1, 6:7], sw[0:1, 4:5])
            nc.vector.tensor_scalar(wvec[0:1, 7:8], sw[0:1, 4:5], 0.0, None,
                                    OP.mult)

            # broadcast scalars to all 128 partitions (gpsimd broadcast)
            bc = cp.tile([128, 8], f32, tag="bc")
            nc.gpsimd.partition_broadcast(bc[:], wvec[:])

            # ---------- q/k/v pipelines (merged double-rms identity) ------
            # a = x0 / sqrt(x0^2/2 + eps*(mean(x^2)+eps)) == rms->rms chain
            x0q, x1q = xq[:, 0:128], xq[:, 128:256]
            cosq, sinq = tq[:, 0:128], tq[:, 128:256]
            k0, k1 = xk[:, 0:128], xk[:, 128:256]
            cosk, sink = tk[:, 0:128], tk[:, 128:256]
            vb0, vb1 = xkb[:, 0:32], xkb[:, 32:64]

            sqq = wp.tile([16, 128], f32, tag="sqq")
            nc.vector.tensor_tensor(sqq[:], x0q, x0q, OP.mult)
            mq = wp.tile([16, 128], f32, tag="mq")
            nc.vector.tensor_tensor(mq[:], x1q, x1q, OP.mult)
            nc.vector.tensor_tensor(mq[:], mq[:], sqq[:], OP.add)
            nc.vector.tensor_scalar(mq[:], mq[:], 0.5 * EPS, EPS * EPS,
                                    OP.mult, OP.add)
            amq = wp.tile([16, 128], f32, tag="amq")
            nc.vector.scalar_tensor_tensor(amq[:], sqq[:], 0.5, mq[:],
                                           OP.mult, OP.add)

            sqk = wp.tile([32, 128], f32, tag="sqk")
            nc.vector.tensor_tensor(sqk[:], k0, k0, OP.mult)
            mk = wp.tile([32, 128], f32, tag="mk")
            nc.vector.tensor_tensor(mk[:], k1, k1, OP.mult)
            nc.vector.tensor_tensor(mk[:], mk[:], sqk[:], OP.add)
            nc.vector.tensor_scalar(mk[:], mk[:], 0.5 * EPS, EPS * EPS,
                                    OP.mult, OP.add)
            amk = wp.tile([32, 128], f32, tag="amk")
            nc.vector.scalar_tensor_tensor(amk[:], sqk[:], 0.5, mk[:],
                                           OP.mult, OP.add)

            mb = wp.tile([128, 32], f32, tag="mb")
            nc.vector.tensor_tensor(mb[:], vb0, vb0, OP.mult)
            bsq1 = wp.tile([128, 32], f32, tag="bsq1")
            nc.vector.tensor_tensor(bsq1[:], vb1, vb1, OP.mult)
            nc.vector.tensor_tensor(mb[:], mb[:], bsq1[:], OP.add)
            nc.vector.tensor_scalar(mb[:], mb[:], 0.5, EPS, OP.mult, OP.add)

            # grouped Sqrt (one table) + DVE reciprocals
            sra = wp.tile([16, 128], f32, tag="sra")
            nc.scalar.activation(sra[:], amq[:], AF.Sqrt)
            srk = wp.tile([32, 128], f32, tag="srk")
            nc.scalar.activation(srk[:], amk[:], AF.Sqrt)
            srb = wp.tile([128, 32], f32, tag="srb")
            nc.scalar.activation(srb[:], mb[:], AF.Sqrt)
            ra = wp.tile([16, 128], f32, tag="ra")
            nc.vector.reciprocal(ra[:], sra[:])
            rak = wp.tile([32, 128], f32, tag="rak")
            nc.vector.reciprocal(rak[:], srk[:])
            rb = wp.tile([128, 32], f32, tag="rb")
            nc.vector.reciprocal(rb[:], srb[:])

            aq = wp.tile([16, 128], f32, tag="aq")
            nc.vector.tensor_tensor(aq[:], x0q, ra[:], OP.mult)
            ak = wp.tile([32, 128], f32, tag="ak")
            nc.vector.tensor_tensor(ak[:], k0, rak[:], OP.mult)
            xn1b = wp.tile([128, 32], f32, tag="xn1b")
            nc.vector.tensor_tensor(xn1b[:], vb1, rb[:], OP.mult)

            qz = wp.tile([16, 128], f32, tag="qzt")
            nc.vector.tensor_scalar(qz[:], aq[:], -1.0, None, OP.mult)
            nc.vector.tensor_tensor(qz[:], qz[:], aq[:], OP.max)
            nc.vector.tensor_scalar(qz[:], qz[:], -SCALE * SQRT2, None, OP.mult)
            t1 = wp.tile([16, 128], f32, tag="t1")
            nc.vector.tensor_scalar(t1[:], cosq, bc[0:16, 0:1], None, OP.mult)
            t2q = wp.tile([16, 128], f32, tag="t2q")
            nc.vector.tensor_scalar(t2q[:], sinq, bc[0:16, 1:2], None, OP.mult)
            nc.vector.tensor_tensor(t1[:], t1[:], t2q[:], OP.add)
            qx = wp.tile([16, 128], f32, tag="qx")
            nc.vector.tensor_tensor(qx[:], aq[:], t1[:], OP.mult)
            t3 = wp.tile([16, 128], f32, tag="t3")
            nc.vector.tensor_scalar(t3[:], sinq, bc[0:16, 0:1], None, OP.mult)
            t4 = wp.tile([16, 128], f32, tag="t4")
            nc.vector.tensor_scalar(t4[:], cosq, bc[0:16, 1:2], None, OP.mult)
            nc.vector.tensor_tensor(t3[:], t3[:], t4[:], OP.subtract)
            qy = wp.tile([16, 128], f32, tag="qy")
            nc.vector.tensor_tensor(qy[:], aq[:], t3[:], OP.mult)

            kx = wp.tile([32, 128], f32, tag="kx")
            nc.vector.tensor_tensor(kx[:], ak[:], cosk, OP.mult)
            ky = wp.tile([32, 128], f32, tag="ky")
            nc.vector.tensor_tensor(ky[:], ak[:], sink, OP.mult)

            # ---------- triple-bf16 splits (separate tiles: packs overlap) --
            def split3(srcl, p, tag):
                out = []
                for a in range(3):
                    l = cp.tile([p, 128], bf16, tag=f"{tag}l{a}",
                                name=f"{tag}l{a}")
                    nc.vector.tensor_copy(l[:], srcl[:])
                    out.append(l)
                    if a < 2:
                        r1 = wp.tile([p, 128], f32, tag=f"{tag}r{a}",
                                     name=f"{tag}r{a}")
                        nc.vector.tensor_tensor(r1[:], srcl[:], l[:],
                                                OP.subtract)
                        srcl = r1
                return out

            qxs = split3(qx, 16, "qx")
            qys = split3(qy, 16, "qy")
            qzs = split3(qz, 16, "qz")
            kxs = split3(kx, 32, "kx")
            kys = split3(ky, 32, "ky")
            konel = cp.tile([32, 128], bf16, tag="konel")
            nc.gpsimd.memset(konel[:], 1.0)
            kzerl = cp.tile([32, 128], bf16, tag="kzerl")
            nc.gpsimd.memset(kzerl[:], 0.0)

            # preload the Exp activation table while splits/packs run
            dummy_exp = cp.tile([1, 1], f32, tag="dummy_exp")
            dummy_sq = cp.tile([1, 1], f32, tag="dummy_sq")
            nc.scalar.activation(dummy_exp[:], sw[0:1, 0:1], AF.Exp)

            # ---------- pack Qop/Kop (row DMAs, 2 queues round-robin) ------
            Qop = cp.tile([18, 2048], bf16, tag="Qop")
            Kop = cp.tile([18, 4096], bf16, tag="Kop")
            engs = [nc.sync, nc.scalar]
            nd_dma = 0

            def pack(dst, row, srcl):
                nonlocal nd_dma
                engs[nd_dma % 2].dma_start(dst[row:row + 1, :], srcl[:])
                nd_dma += 1

            for i, a in enumerate(QLVL):
                pack(Qop, 3 * i, qxs[a])
                pack(Qop, 3 * i + 1, qys[a])
                pack(Qop, 3 * i + 2, qzs[a])
            for i, bl in enumerate(KLVL):
                pack(Kop, 3 * i, kxs[bl])
                pack(Kop, 3 * i + 1, kys[bl])
                pack(Kop, 3 * i + 2, konel if bl == 0 else kzerl)

            # ---------- v values block-major ([128,32], block g) ----------
            vv = wp.tile([128, 32], f32, tag="vv")
            nc.vector.tensor_scalar(vv[:], xn1b[:], bc[:, 2:3], None, OP.mult)
            # vbt cols per block g: [vh | vl | ones] at 3g..3g+2, all f32r
            vbt = cp.tile([128, 96], f32r, tag="vbt")
            nc.vector.tensor_copy(vbt[:, 0:96:3], vv[:])
            vlo = wp.tile([128, 32], f32, tag="vlo")
            nc.vector.tensor_tensor(vlo[:], vv[:], vbt[:, 0:96:3], OP.subtract)
            nc.vector.tensor_copy(vbt[:, 1:96:3], vlo[:])
            onesf = cp.tile([128, 32], f32, tag="onesf")
            nc.gpsimd.memset(onesf[:], 1.0)
            nc.vector.tensor_copy(vbt[:, 2:96:3], onesf[:])

            # ---------- main attention loop (software-pipelined) ----------
            # NDall: per chunk slot s, [3,512] N/D psum rows land in
            # partitions 32s..32s+31 as 16-wide col groups Nh|Nl|D.
            NDall = cp.tile([128, 48], f32, tag="NDall")
            outt = cp.tile([128, 32], f32, tag="outt")

            # persistent finalize temporaries, sliced per slot
            fin = {}
            for t in ("Nrm", "rD", "o0", "h1", "hsq0", "hsq1", "mh", "mh_ln",
                      "mh_r", "hn0", "hn1", "g0", "gt", "g1", "s0e", "s0r",
                      "s0s", "s1e", "s1r", "s1s", "df"):
                fin[t] = cp.tile([128, 16], f32, tag="fin_" + t,
                                 name="fin_" + t)

            def finalize_slot_vec(s):
                # per-slot vector-only phase: o0, h1, mh (no scalar queue use)
                r0, r1 = 32 * s, 32 * s + 32

                def S(t):
                    return fin[t][r0:r1, :]

                x0s, x1s = xq2[r0:r1, 0:16], xq2[r0:r1, 16:32]
                nc.vector.tensor_tensor(S("Nrm"), NDall[r0:r1, 0:16],
                                        NDall[r0:r1, 16:32], OP.add)
                nc.vector.reciprocal(S("rD"), NDall[r0:r1, 32:48])
                nc.vector.tensor_tensor(S("o0"), S("Nrm"), S("rD"), OP.mult)
                nc.vector.tensor_tensor(S("h1"), x1s, S("o0"), OP.add)
                nc.vector.tensor_tensor(S("hsq0"), x0s, x0s, OP.mult)
                nc.vector.tensor_tensor(S("hsq1"), S("h1"), S("h1"), OP.mult)
                nc.vector.tensor_tensor(S("mh"), S("hsq0"), S("hsq1"), OP.add)
                nc.vector.tensor_scalar(S("mh"), S("mh"), 0.5, EPS,
                                        OP.mult, OP.add)

            def finalize_tail():
                # full-width epilogue after all slots
                def S(t):
                    return fin[t][:]

                x0s, x1s = xq2[:, 0:16], xq2[:, 16:32]
                nc.scalar.activation(S("mh_ln"), S("mh"), AF.Sqrt)
                # preload Exp table (silu) while the vector gates run
                nc.scalar.activation(dummy_exp[:], sw[0:1, 0:1], AF.Exp)
                nc.vector.reciprocal(S("mh_r"), S("mh_ln"))
                nc.vector.tensor_tensor(S("hn0"), x0s, S("mh_r"), OP.mult)
                nc.vector.tensor_tensor(S("hn1"), S("h1"), S("mh_r"), OP.mult)
                nc.vector.tensor_scalar(S("g0"), S("hn0"), bc[:, 3:4],
                                        None, OP.mult)
                nc.vector.tensor_scalar(S("gt"), S("hn1"), bc[:, 4:5],
                                        None, OP.mult)
                nc.vector.tensor_tensor(S("g0"), S("g0"), S("gt"), OP.add)
                nc.vector.tensor_scalar(S("g1"), S("hn0"), bc[:, 5:6],
                                        None, OP.mult)
                nc.vector.tensor_tensor(S("g1"), S("g1"), S("gt"), OP.add)
                for gg, nm in (("g0", "s0"), ("g1", "s1")):
                    nc.scalar.activation(S(nm + "e"), S(gg), AF.Exp,
                                         scale=-1.0)
                    nc.vector.tensor_scalar(S(nm + "e"), S(nm + "e"), 1.0,
                                            None, OP.add)
                    nc.vector.reciprocal(S(nm + "r"), S(nm + "e"))
                    nc.vector.tensor_tensor(S(nm + "s"), S(gg), S(nm + "r"),
                                            OP.mult)
                nc.vector.tensor_tensor(S("df"), S("s1s"), S("s0s"),
                                        OP.subtract)
                nc.vector.tensor_tensor(S("df"), S("df"), S("hn0"), OP.mult)
                nc.vector.tensor_scalar(S("df"), S("df"), bc[:, 6:7],
                                        None, OP.mult)
                nc.vector.tensor_copy(outt[:, 0:16], x0s)
                nc.vector.tensor_tensor(outt[:, 16:32], S("h1"), S("df"),
                                        OP.add)

            for s in (3, 2, 1, 0):  # big slots first: tail = smallest slot
                P = PPAIRS[s]
                nd_ps = psnd.tile([3, 512], f32)
                e_tiles = {}
                for jj in range(P + 2):
                    if jj < P:
                        j = jj
                        ps = pslog.tile([128, 1024], f32)
                        nc.tensor.matmul(ps[:, 0:512],
                                         Kop[:, 256 * j:256 * j + 128],
                                         Qop[:, 512 * s:512 * (s + 1)],
                                         start=True, stop=True)
                        nc.tensor.matmul(ps[:, 512:1024],
                                         Kop[:, 256 * j + 128:256 * j + 256],
                                         Qop[:, 512 * s:512 * (s + 1)],
                                         start=True, stop=True)
                        e = ep.tile([128, 1024], f32r)
                        nc.scalar.activation(e[:], ps[:], AF.Exp)
                        if j >= P - NMASK:
                            m = 4 * s + (j - (P - NMASK))
                            nc.vector.tensor_tensor(
                                e[:], e[:],
                                masks[:, 1024 * m:1024 * (m + 1)], OP.mult)
                        e_tiles[j] = e
                    if jj >= 2:
                        j = jj - 2
                        e = e_tiles.pop(j)
                        nc.tensor.matmul(nd_ps[:], vbt[:, 6 * j:6 * j + 3],
                                         e[:, 0:512],
                                         start=(j == 0), stop=False)
                        nc.tensor.matmul(nd_ps[:],
                                         vbt[:, 6 * j + 3:6 * j + 6],
                                         e[:, 512:1024],
                                         start=False, stop=(j == P - 1))
                nds = wp.tile([3, 512], f32, tag="nds")
                nc.vector.tensor_copy(nds[:], nd_ps[:])
                nc.sync.dma_start(NDall[32 * s:32 * s + 32, 0:16],
                                  nds[0:1, :])
                nc.sync.dma_start(NDall[32 * s:32 * s + 32, 16:32],
                                  nds[1:2, :])
                nc.sync.dma_start(NDall[32 * s:32 * s + 32, 32:48],
                                  nds[2:3, :])
                finalize_slot_vec(s)
                if s == 0:
                    # preload Sqrt table under the NDall DMA window
                    nc.scalar.activation(dummy_sq[:], sw[0:1, 0:1], AF.Sqrt)

            finalize_tail()
            nc.sync.dma_start(out_d[:], outt[:])

    nc.compile()
    return nc


def _host_inputs(x, mask, q_weight, v_weight, gate_weight, carry_weight):
    """Build the 8 per-core input maps. Host work is layout/indexing only."""
    f32 = np.float32
    x = np.ascontiguousarray(x, dtype=f32)
    theta = np.arange(L, dtype=f32) * f32(OMEGA)
    cth = np.cos(theta).astype(f32)
    sth = np.sin(theta).astype(f32)

    kk = np.arange(128)[:, None]
    qq = np.arange(512)[None, :]

    def rm(a0, a1, p, w):
        out = np.empty((p, 2 * w), f32)
        out[:, 0:w] = a0.reshape(p, w)
        out[:, w:2 * w] = a1.reshape(p, w)
        return out

    tk = rm(cth, sth, 32, 128)

    in_maps = []
    for core in range(N_CORES):
        b, h = core // 2, core % 2
        chunks = CHUNKS[h]
        # query-side local pos over 4 slots of 512
        qpos = np.concatenate([np.arange(512) + 512 * C for C in chunks])
        # pipeline layout [16, 128], pos = 128p+m
        xq = rm(x[b, qpos, 0], x[b, qpos, 1], 16, 128)
        tq = rm(cth[qpos], sth[qpos], 16, 128)
        # finalize layout [128, 16], pos = 16p+m
        xq2 = rm(x[b, qpos, 0], x[b, qpos, 1], 128, 16)
        # key-side [32, 128], t = 128p+n
        xk = rm(x[b, :, 0], x[b, :, 1], 32, 128)
        # block-major: xkb[p, g] = x[128g+p]
        xkb = np.empty((128, 64), f32)
        xkb[:, 0:32] = x[b, :, 0].reshape(32, 128).T
        xkb[:, 32:64] = x[b, :, 1].reshape(32, 128).T
        # mask blob: per slot s, the last NMASK static pairs are masked.
        m = np.empty((128, 16 * 1024), np.float32)
        for s, C in enumerate(chunks):
            P = PPAIRS[s]
            for j4 in range(NMASK):
                j = P - NMASK + j4
                for side in range(2):
                    g = 2 * j + side
                    col = (4 * s + j4) * 1024 + side * 512
                    m[:, col:col + 512] = (128 * g + kk <= 512 * C + qq)
        in_maps.append({
            "xq": xq, "tq": tq, "xq2": xq2, "xk": xk, "tk": tk, "xkb": xkb,
            "masks": m.astype(np.dtype("bfloat16") if False else f32),
            "wq": np.asarray(q_weight, f32),
            "wv": np.asarray(v_weight, f32),
            "wg": np.asarray(gate_weight, f32),
            "wc": np.asarray(carry_weight, f32),
        })
    # convert masks to bf16 via ml_dtypes
    import ml_dtypes
    for im in in_maps:
        im["masks"] = im["masks"].astype(ml_dtypes.bfloat16)
    return in_maps


def kernel(x, mask, q_weight, v_weight, gate_weight, carry_weight,
           _want_results=False):
    global _compiled
    from concourse.bass_utils import run_bass_kernel_spmd

    if _compiled is None:
        _compiled = _build()
    in_maps = _host_inputs(x, mask, q_weight, v_weight, gate_weight,
                           carry_weight)
    res = run_bass_kernel_spmd(_compiled, in_maps, list(range(N_CORES)))
    out = np.empty((B, L, 2), np.float32)
    for b in range(B):
        for h in range(2):
            r = res.results[2 * b + h]["out"]  # [128, 32]
            ch0 = r[:, 0:16].reshape(-1)
            ch1 = r[:, 16:32].reshape(-1)
            for s, C in enumerate(CHUNKS[h]):
                out[b, 512 * C:512 * (C + 1), 0] = ch0[512 * s:512 * (s + 1)]
                out[b, 512 * C:512 * (C + 1), 1] = ch1[512 * s:512 * (s + 1)]
    if _want_results:
        return out, res
    return out
